# revision 1
# baseline (speedup 1.0000x reference)
"""Trainium2 Bass kernel for the RNN-T style Joiner:
    out = softmax((enc[b,t,:] + dec[b,u,:]) @ W.T + b)  over vocab V

Key algebraic factoring: (enc+dec) @ W.T = enc@W.T [T,V] + dec@W.T [U,V],
so the huge [B,T,U,H] einsum collapses to two small matmuls plus a
broadcast-add, which the PE performs directly into PSUM via selection
matmuls. Softmax over V=128 is done in a [t-partition, (u,v)-free] layout
so the row-sum is a free-dim segmented reduce on DVE.

Sharding: data-parallel over B=8, one batch element per NeuronCore.
"""

import sys

sys.path.insert(0, "/opt/trn_rl_repo")

import numpy as np

B, T, U, H, V = 8, 256, 64, 1024, 128
NCORES = 8
P = 128          # partitions
HC = H // P      # 8 h-chunks of 128
TT = T // P      # 2 t-tiles of 128
UQ = 4           # u's per chunk (4*128 = 512 = max matmul free dim / PSUM bank)
NCH = U // UQ    # 16 chunks per t-tile

_CACHE = {}


def _build():
    from contextlib import ExitStack

    import concourse.bass as bass  # noqa: F401
    import concourse.tile as tile
    from concourse import bacc, mybir

    f32 = mybir.dt.float32
    nc = bacc.Bacc("TRN2", target_bir_lowering=False, debug=False,
                   num_devices=NCORES)

    encT = nc.dram_tensor("encT", [H, T], f32, kind="ExternalInput").ap()
    decT = nc.dram_tensor("decT", [H, U], f32, kind="ExternalInput").ap()
    WT = nc.dram_tensor("WT", [H, V], f32, kind="ExternalInput").ap()
    biasr = nc.dram_tensor("biasr", [1, V], f32, kind="ExternalInput").ap()
    R1 = nc.dram_tensor("R1", [V, UQ * V], f32, kind="ExternalInput").ap()
    ones = nc.dram_tensor("ones", [1, P], f32, kind="ExternalInput").ap()
    out = nc.dram_tensor("out", [T, U, V], f32, kind="ExternalOutput").ap()

    with tile.TileContext(nc) as tc, ExitStack() as ctx:
        const = ctx.enter_context(tc.tile_pool(name="const", bufs=1))
        psum_prep = ctx.enter_context(
            tc.tile_pool(name="psum_prep", bufs=1, space="PSUM"))
        psum_z = ctx.enter_context(
            tc.tile_pool(name="psum_z", bufs=4, space="PSUM"))
        work = ctx.enter_context(tc.tile_pool(name="work", bufs=4))

        # ---- load inputs (h on partitions for all matmul operands) ----
        sb_encT = const.tile([P, HC, T], f32)
        nc.sync.dma_start(out=sb_encT[:],
                          in_=encT.rearrange("(c p) t -> p c t", p=P))
        sb_decT = const.tile([P, HC, U], f32)
        nc.sync.dma_start(out=sb_decT[:],
                          in_=decT.rearrange("(c p) u -> p c u", p=P))
        sb_WT = const.tile([P, HC, V], f32)
        nc.sync.dma_start(out=sb_WT[:],
                          in_=WT.rearrange("(c p) v -> p c v", p=P))
        sb_bias = const.tile([1, V], f32)
        nc.sync.dma_start(out=sb_bias[:], in_=biasr)
        sb_R1 = const.tile([P, UQ * V], f32)
        nc.sync.dma_start(out=sb_R1[:], in_=R1)
        sb_ones = const.tile([1, P], f32)
        nc.sync.dma_start(out=sb_ones[:], in_=ones)

        # ---- ET[v, t] = (enc @ W.T).T : accumulate over h-chunks ----
        ps_ET = psum_prep.tile([P, T], f32)
        for c in range(HC):
            nc.tensor.matmul(ps_ET[:], lhsT=sb_WT[:, c, :],
                             rhs=sb_encT[:, c, :],
                             start=(c == 0), stop=(c == HC - 1))
        sb_ET = const.tile([P, T], f32)
        nc.vector.tensor_copy(out=sb_ET[:], in_=ps_ET[:])

        # ---- Dp[u, v] = dec @ W.T + bias ----
        ps_Dp = psum_prep.tile([U, V], f32)
        for c in range(HC):
            nc.tensor.matmul(ps_Dp[:], lhsT=sb_decT[:, c, :],
                             rhs=sb_WT[:, c, :],
                             start=(c == 0), stop=False)
        # + bias broadcast to all u partitions via ones-column
        nc.tensor.matmul(ps_Dp[:], lhsT=sb_ones[0:1, 0:U], rhs=sb_bias[:],
                         start=False, stop=True)
        sb_Dp = const.tile([U, V], f32)
        nc.vector.tensor_copy(out=sb_Dp[:], in_=ps_Dp[:])
        # flatten [U, V] -> [1, U*V] (cross-partition) so a K=1 matmul can
        # broadcast Dp rows across all t partitions
        sb_Dpflat = const.tile([1, U * V], f32)
        nc.sync.dma_start(out=sb_Dpflat[:], in_=sb_Dp[:])

        # ---- main: 2 t-tiles x 16 u-quad chunks ----
        for tt in range(TT):
            for ck in range(NCH):
                # logits chunk Z[t, (u, v)] = E[t, v] + Dp[u, v] in PSUM
                ps = psum_z.tile([P, UQ * V], f32, tag="z")
                nc.tensor.matmul(ps[:], lhsT=sb_ET[:, tt * P:(tt + 1) * P],
                                 rhs=sb_R1[:], start=True, stop=False)
                nc.tensor.matmul(
                    ps[:], lhsT=sb_ones[0:1, :],
                    rhs=sb_Dpflat[0:1, ck * UQ * V:(ck + 1) * UQ * V],
                    start=False, stop=True)

                # exp (PSUM -> SBUF)
                p_sb = work.tile([P, UQ * V], f32, tag="p")
                nc.scalar.activation(p_sb[:], ps[:],
                                     mybir.ActivationFunctionType.Exp)

                # denominator: segmented sum over v per (t, u)
                s_sb = work.tile([P, UQ], f32, tag="s")
                nc.vector.tensor_reduce(
                    out=s_sb[:],
                    in_=p_sb[:].rearrange("p (a b) -> p a b", a=UQ),
                    axis=mybir.AxisListType.X, op=mybir.AluOpType.add)
                r_sb = work.tile([P, UQ], f32, tag="r")
                nc.vector.reciprocal(out=r_sb[:], in_=s_sb[:])

                # normalize
                o_sb = work.tile([P, UQ, V], f32, tag="o")
                nc.vector.tensor_mul(
                    o_sb[:],
                    p_sb[:].rearrange("p (a b) -> p a b", a=UQ),
                    r_sb[:, :, None].broadcast_to([P, UQ, V]))

                nc.sync.dma_start(
                    out=out[tt * P:(tt + 1) * P, ck * UQ:(ck + 1) * UQ, :],
                    in_=o_sb[:])

    nc.compile()
    return nc


def _get_nc():
    if "nc" not in _CACHE:
        _CACHE["nc"] = _build()
    return _CACHE["nc"]


def _make_in_maps(enc, dec, W, b):
    WT = np.ascontiguousarray(W.T)                       # [H, V]
    biasr = np.ascontiguousarray(b.reshape(1, V))
    R1 = np.tile(np.eye(V, dtype=np.float32), (1, UQ))   # [V, UQ*V]
    ones = np.ones((1, P), dtype=np.float32)
    maps = []
    for i in range(NCORES):
        maps.append({
            "encT": np.ascontiguousarray(enc[i].T),      # [H, T]
            "decT": np.ascontiguousarray(dec[i].T),      # [H, U]
            "WT": WT, "biasr": biasr, "R1": R1, "ones": ones,
        })
    return maps


def kernel(outputs_encoder, outputs_decoder, W, b):
    enc = np.asarray(outputs_encoder, dtype=np.float32)
    dec = np.asarray(outputs_decoder, dtype=np.float32)
    W = np.asarray(W, dtype=np.float32)
    b = np.asarray(b, dtype=np.float32)

    from concourse.bass_utils import run_bass_kernel_spmd

    nc = _get_nc()
    in_maps = _make_in_maps(enc, dec, W, b)
    res = run_bass_kernel_spmd(nc, in_maps, list(range(NCORES)))
    out = np.stack([np.asarray(res.results[i]["out"]) for i in range(NCORES)])
    return out.astype(np.float32)
